# revision 15
# baseline (speedup 1.0000x reference)
"""Causal self-attention layer on 8 Trainium2 NeuronCores.

Reference computation (fp32):
    kqv = x @ W_kqv.T + b_kqv ; split q,k,v ; per-head causal softmax attention
    y = att @ v ; out = y @ W_proj.T + b_proj
Shapes: x [4, 2048, 1024], 16 heads, head_dim 64.

Sharding: 8 cores = 4 batches x 2 head-groups (8 heads each). Each core
computes QKV for its 8 heads, causal attention, and a partial output
projection (contraction over its 512 y-columns). Host sums the two
partials per batch and adds b_proj.

Device layout notes:
 - Attention is computed transposed: logitsT[j, i] = sum_d k[j,d] q[i,d],
   so attT [j(part), i(free)] feeds att@V directly as the moving operand
   and the softmax denominator comes from a ones-column appended to V
   (weight column of zeros + bias 1.0), with no on-chip transposes.
 - 1/sqrt(head_dim) is folded into W_k / b_k on the host.
 - Matmul operands are bitcast to float32r (full-rate fp32 on the PE).
"""

import numpy as np

B, S, E = 4, 2048, 1024
H, D = 16, 64
P = 128
HL = 8              # heads per core
FL = HL * D         # 512 local f columns
VW = HL * (D + 1)   # 520: v columns incl. per-head ones column
ECH = E // P        # 8 contraction chunks
PAIRS = HL // 2     # 4 head pairs
NSB = S // P        # 16 s-blocks
NIC = S // 512      # 4 i-chunks

_CACHE = {}


def _build_nc():
    import concourse.bacc as bacc
    import concourse.tile as tile
    from concourse import mybir
    from contextlib import ExitStack

    f32 = mybir.dt.float32
    f32r = mybir.dt.float32r
    FT = mybir.ActivationFunctionType

    nc = bacc.Bacc("TRN2", target_bir_lowering=False, debug=False)
    xT_d = nc.dram_tensor("xT", [E, S], f32r, kind="ExternalInput").ap()
    wqk_d = nc.dram_tensor("wqkT", [E, 2 * FL], f32r, kind="ExternalInput").ap()
    bqk_d = nc.dram_tensor("bqk", [2 * FL], f32, kind="ExternalInput").ap()
    wv_d = nc.dram_tensor("wvT", [E, VW], f32r, kind="ExternalInput").ap()
    bvr_d = nc.dram_tensor("bvrep", [P, VW], f32r, kind="ExternalInput").ap()
    wp_d = nc.dram_tensor("wpT", [FL, E], f32r, kind="ExternalInput").ap()
    mask_d = nc.dram_tensor("mask", [P, P], f32r, kind="ExternalInput").ap()
    out_d = nc.dram_tensor("out", [S, E], f32, kind="ExternalOutput").ap()

    with tile.TileContext(nc) as tc, ExitStack() as ctx:
        # pools that live for the whole kernel
        consts = ctx.enter_context(tc.tile_pool(name="consts", bufs=1))
        qkpool = ctx.enter_context(tc.tile_pool(name="qkp", bufs=1))
        vpool = ctx.enter_context(tc.tile_pool(name="vp", bufs=1))


        # ---- constants ----
        mask_sb = consts.tile([P, P], f32r, name="mask_sb")
        nc.sync.dma_start(out=mask_sb, in_=mask_d)
        bqk_sb = consts.tile([P, ECH], f32, name="bqk_sb")
        for t in range(ECH):
            nc.sync.dma_start(
                out=bqk_sb[:, t : t + 1],
                in_=bqk_d[P * t : P * (t + 1)].unsqueeze(1),
            )
        bvr_sb = consts.tile([P, VW], f32r, name="bvr_sb")
        nc.sync.dma_start(out=bvr_sb, in_=bvr_d)

        v_sb = [vpool.tile([P, VW], f32r, name=f"v{sb}", tag=f"v{sb}") for sb in range(NSB)]
        qk_sb = [qkpool.tile([P, S], f32r, name=f"qk{t}", tag=f"qk{t}") for t in range(ECH)]

        with (
            tc.tile_pool(name="xp", bufs=1) as xpool,
            tc.tile_pool(name="mmq", bufs=2, space="PSUM") as mmps,
        ):
            xT_sb = []
            for c in range(ECH):
                t_ = xpool.tile([P, S], f32r, name=f"x{c}", tag=f"x{c}")
                nc.sync.dma_start(out=t_, in_=xT_d[P * c : P * (c + 1), :])
                xT_sb.append(t_)

            # ---- QKV: v tiles, [s(part) x f] layout incl. ones columns ----
            with tc.tile_pool(name="wvp", bufs=1) as wvpool:
                wv_sb = []
                for c in range(ECH):
                    t_ = wvpool.tile([P, VW], f32r, name=f"wv{c}", tag=f"wv{c}")
                    nc.sync.dma_start(out=t_, in_=wv_d[P * c : P * (c + 1), :])
                    wv_sb.append(t_)
                for sb in range(NSB):
                    for fc in range(2):
                        f0, f1 = fc * 260, min(VW, (fc + 1) * 260)
                        ps = mmps.tile([P, 512], f32, name="mm", tag="mm")
                        for ec in range(ECH):
                            nc.tensor.matmul(
                                ps[:, : f1 - f0],
                                lhsT=xT_sb[ec][:, P * sb : P * (sb + 1)],
                                rhs=wv_sb[ec][:, f0:f1],
                                start=(ec == 0),
                                stop=(ec == ECH - 1),
                            )
                        nc.vector.tensor_add(
                            out=v_sb[sb][:, f0:f1],
                            in0=ps[:, : f1 - f0],
                            in1=bvr_sb[:, f0:f1],
                        )

            # ---- QKV: q and k tiles, [f(part) x s] layout, pairs stacked ----
            with tc.tile_pool(name="wqkp", bufs=2) as wqkp:
                for t in range(ECH):  # t<4: q pair t ; t>=4: k pair t-4
                    dst = qk_sb[t]
                    wt = wqkp.tile([P, ECH, P], f32r, name="wqk", tag="wqk")
                    nc.sync.dma_start(
                        out=wt,
                        in_=wqk_d[:, P * t : P * (t + 1)].rearrange(
                            "(c p) f -> p c f", p=P
                        ),
                    )
                    for sc in range(4):
                        ps = mmps.tile([P, 512], f32, name="mm", tag="mm")
                        for ec in range(ECH):
                            nc.tensor.matmul(
                                ps,
                                lhsT=wt[:, ec, :],
                                rhs=xT_sb[ec][:, 512 * sc : 512 * (sc + 1)],
                                start=(ec == 0),
                                stop=(ec == ECH - 1),
                            )
                        nc.vector.tensor_scalar_add(
                            out=dst[:, 512 * sc : 512 * (sc + 1)],
                            in0=ps,
                            scalar1=bqk_sb[:, t : t + 1],
                        )

        # ---- attention (transposed), per head pair ----
        ytpool = ctx.enter_context(tc.tile_pool(name="ytp", bufs=1))
        yt_sb = [
            ytpool.tile([P, S], f32r, name=f"yt{pr}", tag=f"yt{pr}")
            for pr in range(PAIRS)
        ]
        with (
            tc.tile_pool(name="atp", bufs=3) as atpool,
            tc.tile_pool(name="yup", bufs=2) as yupool,
            tc.tile_pool(name="bcdp", bufs=2) as bcdpool,
            tc.tile_pool(name="lgps", bufs=2, space="PSUM") as lgps,
            tc.tile_pool(name="ypsp", bufs=4, space="PSUM") as ypsp,
        ):
            for pr in range(PAIRS):
                qtile, ktile = qk_sb[pr], qk_sb[4 + pr]
                yu = [
                    yupool.tile([D + 1, S], f32, name="yuA", tag="yuA"),
                    yupool.tile([D + 1, S], f32, name="yuB", tag="yuB"),
                ]
                for ic in range(NIC):
                    njb = 4 * ic + 4
                    yps = [
                        ypsp.tile([D + 1, 512], f32, name="yps", tag="yps"),
                        ypsp.tile([D + 1, 512], f32, name="yps", tag="yps"),
                    ]
                    for jb in range(njb):
                        off = max(0, P * (jb - 4 * ic))
                        lg = lgps.tile([P, 2, 512], f32, name="lg", tag="lg")
                        for hi in range(2):
                            r0 = D * hi
                            nc.tensor.matmul(
                                lg[:, hi, off:512],
                                lhsT=ktile[
                                    r0 : r0 + D, P * jb : P * (jb + 1)
                                ],
                                rhs=qtile[
                                    r0 : r0 + D, 512 * ic + off : 512 * (ic + 1)
                                ],
                                start=True,
                                stop=True,
                            )
                        at = atpool.tile([P, 2, 512], f32r, name="at", tag="at")
                        nc.scalar.activation(
                            at[:, :, off:512], lg[:, :, off:512], FT.Exp
                        )
                        if jb >= 4 * ic:  # diagonal block: causal mask
                            for hi in range(2):
                                nc.vector.tensor_mul(
                                    at[:, hi, off : off + P],
                                    at[:, hi, off : off + P],
                                    mask_sb,
                                )
                        for hi in range(2):
                            h = 2 * pr + hi
                            nc.tensor.matmul(
                                yps[hi][:, off:512],
                                lhsT=v_sb[jb][
                                    :, (D + 1) * h : (D + 1) * (h + 1)
                                ],
                                rhs=at[:, hi, off:512],
                                start=(jb == 0),
                                stop=(jb == njb - 1),
                            )
                    # evict unnormalized yT (+ denom row) to SBUF, release psum
                    for hi in range(2):
                        nc.vector.tensor_copy(
                            out=yu[hi][:, 512 * ic : 512 * (ic + 1)],
                            in_=yps[hi],
                        )
                # bulk normalize per pair: yT = yT_unnorm * bcast(1/denom)
                for hi in range(2):
                    bcd = bcdpool.tile([D, S], f32, name="bcd", tag="bcd")
                    nc.gpsimd.partition_broadcast(bcd, yu[hi][D : D + 1, :])
                    nc.vector.reciprocal(bcd, bcd)
                    nc.vector.tensor_mul(
                        yt_sb[pr][D * hi : D * (hi + 1), :],
                        yu[hi][0:D, :],
                        bcd,
                    )

        # ---- output projection (partial over local heads) ----
        with (
            tc.tile_pool(name="wpp", bufs=1) as wppool,
            tc.tile_pool(name="outp", bufs=3) as opool,
            tc.tile_pool(name="mmo", bufs=2, space="PSUM") as mmps,
        ):
            wp_sb = []
            for c in range(FL // P):
                t_ = wppool.tile([P, E], f32r, name=f"wp{c}", tag=f"wp{c}")
                nc.sync.dma_start(out=t_, in_=wp_d[P * c : P * (c + 1), :])
                wp_sb.append(t_)
            for sb in range(NSB):
                for fc in range(2):
                    ps = mmps.tile([P, 512], f32, name="mm", tag="mm")
                    for kc in range(FL // P):
                        nc.tensor.matmul(
                            ps,
                            lhsT=yt_sb[kc][:, P * sb : P * (sb + 1)],
                            rhs=wp_sb[kc][:, 512 * fc : 512 * (fc + 1)],
                            start=(kc == 0),
                            stop=(kc == FL // P - 1),
                        )
                    ot = opool.tile([P, 512], f32, name="ot", tag="ot")
                    nc.vector.tensor_copy(ot, ps)
                    nc.sync.dma_start(
                        out=out_d[P * sb : P * (sb + 1), 512 * fc : 512 * (fc + 1)],
                        in_=ot,
                    )

    nc.compile()
    return nc


def get_nc():
    if "nc" not in _CACHE:
        _CACHE["nc"] = _build_nc()
    return _CACHE["nc"]


def make_in_maps(x, W_kqv, b_kqv, W_proj, b_proj):
    x = np.asarray(x, np.float32)
    W_kqv = np.asarray(W_kqv, np.float32)
    b_kqv = np.asarray(b_kqv, np.float32)
    W_proj = np.asarray(W_proj, np.float32)
    mask = np.triu(np.ones((P, P), np.float32))  # mask[j, i] = 1 iff j <= i
    scale = np.float32(1.0 / np.sqrt(D))
    in_maps = []
    for core in range(8):
        b, hg = core // 2, core % 2
        r = slice(FL * hg, FL * (hg + 1))
        Wq = W_kqv[0:E][r]
        Wk = W_kqv[E : 2 * E][r] * scale
        Wv = W_kqv[2 * E : 3 * E][r]
        bq = b_kqv[0:E][r]
        bk = b_kqv[E : 2 * E][r] * scale
        bv = b_kqv[2 * E : 3 * E][r]
        WvP = np.zeros((VW, E), np.float32)
        bvP = np.zeros(VW, np.float32)
        for h in range(HL):
            WvP[(D + 1) * h : (D + 1) * h + D] = Wv[D * h : D * (h + 1)]
            bvP[(D + 1) * h : (D + 1) * h + D] = bv[D * h : D * (h + 1)]
            bvP[(D + 1) * h + D] = 1.0
        in_maps.append(
            {
                "xT": np.ascontiguousarray(x[b].T),
                "wqkT": np.ascontiguousarray(np.concatenate([Wq, Wk], 0).T),
                "bqk": np.concatenate([bq, bk]),
                "wvT": np.ascontiguousarray(WvP.T),
                "bvrep": np.tile(bvP[None, :], (P, 1)),
                "wpT": np.ascontiguousarray(W_proj[:, r].T),
                "mask": mask,
            }
        )
    return in_maps


def gather(results, b_proj):
    b_proj = np.asarray(b_proj, np.float32)
    out = np.empty((B, S, E), np.float32)
    for b in range(B):
        out[b] = results[2 * b]["out"] + results[2 * b + 1]["out"] + b_proj
    return out


def run_on_hw(in_maps, trace=False):
    from concourse import bass_utils

    nc = get_nc()
    return bass_utils.run_bass_kernel_spmd(
        nc, in_maps, core_ids=list(range(8)), trace=trace
    )


def kernel(x, W_kqv, b_kqv, W_proj, b_proj):
    in_maps = make_in_maps(x, W_kqv, b_kqv, W_proj, b_proj)
    bkr = run_on_hw(in_maps)
    return gather(bkr.results, b_proj)


# revision 19
# speedup vs baseline: 1.0797x; 1.0797x over previous
"""Causal self-attention layer on 8 Trainium2 NeuronCores.

Reference computation (fp32):
    kqv = x @ W_kqv.T + b_kqv ; split q,k,v ; per-head causal softmax attention
    y = att @ v ; out = y @ W_proj.T + b_proj
Shapes: x [4, 2048, 1024], 16 heads, head_dim 64.

Sharding: 8 cores = 4 batches x 2 head-groups (8 heads each). Each core
computes QKV for its 8 heads, causal attention, and a partial output
projection (contraction over its 512 y-columns). Host sums the two
partials per batch and adds b_proj.

Device layout notes:
 - Attention is computed transposed: logitsT[j, i] = sum_d k[j,d] q[i,d],
   so attT [j(part), i(free)] feeds att@V directly as the moving operand
   and the softmax denominator comes from a ones-column appended to V
   (weight column of zeros + bias 1.0), with no on-chip transposes.
 - 1/sqrt(head_dim) is folded into W_k / b_k on the host.
 - Matmul operands are bitcast to float32r (full-rate fp32 on the PE).
"""

import numpy as np

B, S, E = 4, 2048, 1024
H, D = 16, 64
P = 128
HL = 8              # heads per core
FL = HL * D         # 512 local f columns
VW = HL * (D + 1)   # 520: v columns incl. per-head ones column
ECH = E // P        # 8 contraction chunks
PAIRS = HL // 2     # 4 head pairs
NSB = S // P        # 16 s-blocks
NIC = S // 512      # 4 i-chunks

_CACHE = {}


def _build_nc():
    import concourse.bacc as bacc
    import concourse.tile as tile
    from concourse import mybir
    from contextlib import ExitStack

    f32 = mybir.dt.float32
    f32r = mybir.dt.float32r
    FT = mybir.ActivationFunctionType

    nc = bacc.Bacc("TRN2", target_bir_lowering=False, debug=False)
    xT_d = nc.dram_tensor("xT", [E, S], f32r, kind="ExternalInput").ap()
    wqk_d = nc.dram_tensor("wqkT", [E, 2 * FL], f32r, kind="ExternalInput").ap()
    bqk_d = nc.dram_tensor("bqk", [2 * FL], f32, kind="ExternalInput").ap()
    wv_d = nc.dram_tensor("wvT", [E, VW], f32r, kind="ExternalInput").ap()
    bvr_d = nc.dram_tensor("bvrep", [P, VW], f32r, kind="ExternalInput").ap()
    wp_d = nc.dram_tensor("wpT", [FL, E], f32r, kind="ExternalInput").ap()
    mask_d = nc.dram_tensor("mask", [P, P], f32r, kind="ExternalInput").ap()
    out_d = nc.dram_tensor("out", [S, E], f32, kind="ExternalOutput").ap()

    with tile.TileContext(nc) as tc, ExitStack() as ctx:
        # pools that live for the whole kernel
        consts = ctx.enter_context(tc.tile_pool(name="consts", bufs=1))
        qkpool = ctx.enter_context(tc.tile_pool(name="qkp", bufs=1))
        vpool = ctx.enter_context(tc.tile_pool(name="vp", bufs=1))


        # ---- constants ----
        mask_sb = consts.tile([P, P], f32r, name="mask_sb")
        nc.sync.dma_start(out=mask_sb, in_=mask_d)
        bqk_sb = consts.tile([P, ECH], f32, name="bqk_sb")
        for t in range(ECH):
            nc.sync.dma_start(
                out=bqk_sb[:, t : t + 1],
                in_=bqk_d[P * t : P * (t + 1)].unsqueeze(1),
            )
        bvr_sb = consts.tile([P, VW], f32r, name="bvr_sb")
        nc.sync.dma_start(out=bvr_sb, in_=bvr_d)

        v_sb = [vpool.tile([P, VW], f32r, name=f"v{sb}", tag=f"v{sb}") for sb in range(NSB)]
        qk_sb = [qkpool.tile([P, S], f32r, name=f"qk{t}", tag=f"qk{t}") for t in range(ECH)]

        with (
            tc.tile_pool(name="xp", bufs=1) as xpool,
            tc.tile_pool(name="mmq", bufs=2, space="PSUM") as mmps,
        ):
            xT_sb = []
            for c in range(ECH):
                t_ = xpool.tile([P, S], f32r, name=f"x{c}", tag=f"x{c}")
                nc.sync.dma_start(out=t_, in_=xT_d[P * c : P * (c + 1), :])
                xT_sb.append(t_)

            # ---- QKV: v tiles, [s(part) x f] layout incl. ones columns ----
            with tc.tile_pool(name="wvp", bufs=1) as wvpool:
                wv_sb = []
                for c in range(ECH):
                    t_ = wvpool.tile([P, VW], f32r, name=f"wv{c}", tag=f"wv{c}")
                    nc.sync.dma_start(out=t_, in_=wv_d[P * c : P * (c + 1), :])
                    wv_sb.append(t_)
                for sb in range(NSB):
                    for fc in range(2):
                        f0, f1 = fc * 260, min(VW, (fc + 1) * 260)
                        ps = mmps.tile([P, 512], f32, name="mm", tag="mm")
                        for ec in range(ECH):
                            nc.tensor.matmul(
                                ps[:, : f1 - f0],
                                lhsT=xT_sb[ec][:, P * sb : P * (sb + 1)],
                                rhs=wv_sb[ec][:, f0:f1],
                                start=(ec == 0),
                                stop=(ec == ECH - 1),
                            )
                        nc.vector.tensor_add(
                            out=v_sb[sb][:, f0:f1],
                            in0=ps[:, : f1 - f0],
                            in1=bvr_sb[:, f0:f1],
                        )

            # ---- QKV: q and k tiles, [f(part) x s] layout, pairs stacked ----
            with tc.tile_pool(name="wqkp", bufs=2) as wqkp:
                for t in range(ECH):  # t<4: q pair t ; t>=4: k pair t-4
                    dst = qk_sb[t]
                    wt = wqkp.tile([P, ECH, P], f32r, name="wqk", tag="wqk")
                    nc.sync.dma_start(
                        out=wt,
                        in_=wqk_d[:, P * t : P * (t + 1)].rearrange(
                            "(c p) f -> p c f", p=P
                        ),
                    )
                    for sc in range(4):
                        ps = mmps.tile([P, 512], f32, name="mm", tag="mm")
                        for ec in range(ECH):
                            nc.tensor.matmul(
                                ps,
                                lhsT=wt[:, ec, :],
                                rhs=xT_sb[ec][:, 512 * sc : 512 * (sc + 1)],
                                start=(ec == 0),
                                stop=(ec == ECH - 1),
                            )
                        nc.vector.tensor_scalar_add(
                            out=dst[:, 512 * sc : 512 * (sc + 1)],
                            in0=ps,
                            scalar1=bqk_sb[:, t : t + 1],
                        )

        # ---- attention (transposed), per head pair ----
        # load W_proj early so the projection can start right after the
        # last pair's normalize
        wppool = ctx.enter_context(tc.tile_pool(name="wpp", bufs=1))
        wp_sb = []
        for c in range(FL // P):
            t_ = wppool.tile([P, E], f32r, name=f"wp{c}", tag=f"wp{c}")
            nc.sync.dma_start(out=t_, in_=wp_d[P * c : P * (c + 1), :])
            wp_sb.append(t_)
        ytpool = ctx.enter_context(tc.tile_pool(name="ytp", bufs=1))
        yt_sb = [
            ytpool.tile([P, S], f32r, name=f"yt{pr}", tag=f"yt{pr}")
            for pr in range(PAIRS)
        ]
        with (
            tc.tile_pool(name="atp", bufs=3) as atpool,
            tc.tile_pool(name="yup", bufs=2) as yupool,
            tc.tile_pool(name="bcdp", bufs=2) as bcdpool,
            tc.tile_pool(name="lgps", bufs=2, space="PSUM") as lgps,
            tc.tile_pool(name="ypsp", bufs=4, space="PSUM") as ypsp,
        ):
            for pr in range(PAIRS):
                qtile, ktile = qk_sb[pr], qk_sb[4 + pr]
                yu = [
                    yupool.tile([D + 1, S], f32, name="yuA", tag="yuA"),
                    yupool.tile([D + 1, S], f32, name="yuB", tag="yuB"),
                ]
                for ic in range(NIC):
                    njb = 4 * ic + 4
                    yps = [
                        ypsp.tile([D + 1, 512], f32, name="yps", tag="yps"),
                        ypsp.tile([D + 1, 512], f32, name="yps", tag="yps"),
                    ]
                    for jb in range(njb):
                        off = max(0, P * (jb - 4 * ic))
                        lg = lgps.tile([P, 2, 512], f32, name="lg", tag="lg")
                        for hi in range(2):
                            r0 = D * hi
                            nc.tensor.matmul(
                                lg[:, hi, off:512],
                                lhsT=ktile[
                                    r0 : r0 + D, P * jb : P * (jb + 1)
                                ],
                                rhs=qtile[
                                    r0 : r0 + D, 512 * ic + off : 512 * (ic + 1)
                                ],
                                start=True,
                                stop=True,
                            )
                        at = atpool.tile([P, 2, 512], f32r, name="at", tag="at")
                        nc.scalar.activation(
                            at[:, :, off:512], lg[:, :, off:512], FT.Exp
                        )
                        if jb >= 4 * ic:  # diagonal block: causal mask
                            for hi in range(2):
                                nc.gpsimd.tensor_mul(
                                    at[:, hi, off : off + P],
                                    at[:, hi, off : off + P],
                                    mask_sb,
                                )
                        for hi in range(2):
                            h = 2 * pr + hi
                            nc.tensor.matmul(
                                yps[hi][:, off:512],
                                lhsT=v_sb[jb][
                                    :, (D + 1) * h : (D + 1) * (h + 1)
                                ],
                                rhs=at[:, hi, off:512],
                                start=(jb == 0),
                                stop=(jb == njb - 1),
                            )
                    # evict unnormalized yT (+ denom row) to SBUF, release psum
                    for hi in range(2):
                        nc.vector.tensor_copy(
                            out=yu[hi][:, 512 * ic : 512 * (ic + 1)],
                            in_=yps[hi],
                        )
                # bulk normalize per pair: yT = yT_unnorm * bcast(1/denom)
                for hi in range(2):
                    nc.vector.reciprocal_approx_fast(
                        out=yu[hi][D : D + 1, :], in_=yu[hi][D : D + 1, :]
                    )
                    bcd = bcdpool.tile([D, S], f32, name="bcd", tag="bcd")
                    nc.gpsimd.partition_broadcast(bcd, yu[hi][D : D + 1, :])
                    nc.vector.tensor_mul(
                        yt_sb[pr][D * hi : D * (hi + 1), :],
                        yu[hi][0:D, :],
                        bcd,
                    )

        # ---- output projection (partial over local heads) ----
        with (
            tc.tile_pool(name="outp", bufs=3) as opool,
            tc.tile_pool(name="mmo", bufs=2, space="PSUM") as mmps,
        ):
            for sb in range(NSB):
                for fc in range(2):
                    ps = mmps.tile([P, 512], f32, name="mm", tag="mm")
                    for kc in range(FL // P):
                        nc.tensor.matmul(
                            ps,
                            lhsT=yt_sb[kc][:, P * sb : P * (sb + 1)],
                            rhs=wp_sb[kc][:, 512 * fc : 512 * (fc + 1)],
                            start=(kc == 0),
                            stop=(kc == FL // P - 1),
                        )
                    ot = opool.tile([P, 512], f32, name="ot", tag="ot")
                    nc.vector.tensor_copy(ot, ps)
                    nc.sync.dma_start(
                        out=out_d[P * sb : P * (sb + 1), 512 * fc : 512 * (fc + 1)],
                        in_=ot,
                    )

    nc.compile()
    return nc


def get_nc():
    if "nc" not in _CACHE:
        _CACHE["nc"] = _build_nc()
    return _CACHE["nc"]


def make_in_maps(x, W_kqv, b_kqv, W_proj, b_proj):
    x = np.asarray(x, np.float32)
    W_kqv = np.asarray(W_kqv, np.float32)
    b_kqv = np.asarray(b_kqv, np.float32)
    W_proj = np.asarray(W_proj, np.float32)
    mask = np.triu(np.ones((P, P), np.float32))  # mask[j, i] = 1 iff j <= i
    scale = np.float32(1.0 / np.sqrt(D))
    in_maps = []
    for core in range(8):
        b, hg = core // 2, core % 2
        r = slice(FL * hg, FL * (hg + 1))
        Wq = W_kqv[0:E][r]
        Wk = W_kqv[E : 2 * E][r] * scale
        Wv = W_kqv[2 * E : 3 * E][r]
        bq = b_kqv[0:E][r]
        bk = b_kqv[E : 2 * E][r] * scale
        bv = b_kqv[2 * E : 3 * E][r]
        WvP = np.zeros((VW, E), np.float32)
        bvP = np.zeros(VW, np.float32)
        for h in range(HL):
            WvP[(D + 1) * h : (D + 1) * h + D] = Wv[D * h : D * (h + 1)]
            bvP[(D + 1) * h : (D + 1) * h + D] = bv[D * h : D * (h + 1)]
            bvP[(D + 1) * h + D] = 1.0
        in_maps.append(
            {
                "xT": np.ascontiguousarray(x[b].T),
                "wqkT": np.ascontiguousarray(np.concatenate([Wq, Wk], 0).T),
                "bqk": np.concatenate([bq, bk]),
                "wvT": np.ascontiguousarray(WvP.T),
                "bvrep": np.tile(bvP[None, :], (P, 1)),
                "wpT": np.ascontiguousarray(W_proj[:, r].T),
                "mask": mask,
            }
        )
    return in_maps


def gather(results, b_proj):
    b_proj = np.asarray(b_proj, np.float32)
    out = np.empty((B, S, E), np.float32)
    for b in range(B):
        out[b] = results[2 * b]["out"] + results[2 * b + 1]["out"] + b_proj
    return out


def run_on_hw(in_maps, trace=False):
    from concourse import bass_utils

    nc = get_nc()
    return bass_utils.run_bass_kernel_spmd(
        nc, in_maps, core_ids=list(range(8)), trace=trace
    )


def kernel(x, W_kqv, b_kqv, W_proj, b_proj):
    in_maps = make_in_maps(x, W_kqv, b_kqv, W_proj, b_proj)
    bkr = run_on_hw(in_maps)
    return gather(bkr.results, b_proj)
